# revision 19
# baseline (speedup 1.0000x reference)
"""Trainium2 Bass kernel for a 2-layer GAT (PyG GATConv-style) on 8 NeuronCores.

v2 strategy (dst-node sharding):
  - Edges (with self-loops) are sorted by dst per core and grouped into
    128-dst windows; windows are paired into gather chunks.
  - Layer 1 gathers per-edge SOURCE FEATURES (not projections): x rows are
    2-packed into 512B table rows and fetched with dma_gather(transpose=True),
    landing as [in_chan, 2, edge]; a copy_predicated pass selects the even/odd
    half per edge (host-known parity).  h = x[src] @ W1ext is then computed
    per 128-edge tile on the PE (lhsT = gathered x columns), which also yields
    per-edge a_s (W1ext carries the att_src/att_dst combination columns).
    This removes the replicated N x 256 projection phase and its 38MB table,
    and halves gather descriptor count (the SWDGE ucode is ~8ns/row and
    freezes the DVE while it runs - fewer rows is the whole game).
  - Accumulation is dst-partition oriented: one matmul per tile
    psNT[dst,260] += oh[e,dst]^T @ [alpha*h(256) | exp(e)(4)], with the
    denominator riding in the last 4 columns.  Per-edge attention scalars are
    batched per window ([128, Tw, 4] slabs), not per tile.
  - oh (edge-partition one-hot) is built on DVE; ohT (dst-partition) on ACT
    via relu(1-(dof-d)^2).  a_d[dst] expands per edge via ohT matmuls.
  - Layer-2 table rows hold two nodes' [h2(64)|a_s|a_d|pad] halves (512B rows,
    2-packed again); one AllGather ships the per-core table to all cores.
"""

import sys

sys.path.insert(0, "/opt/trn_rl_repo")

import numpy as np

import concourse.bacc as bacc
import concourse.bass as bass
import concourse.mybir as mybir
import concourse.tile as tile
from concourse import bass_utils

F16 = mybir.dt.float16
F32 = mybir.dt.float32
I16 = mybir.dt.int16
U8 = mybir.dt.uint8
OP = mybir.AluOpType
ACT = mybir.ActivationFunctionType

NEG_SLOPE = 0.2
C_SHIFT = 2.0  # global softmax shift: exp(e - C) - cancels in the ratio


def _midb(ap2d, T):
    """[128, X] AP -> [128, T, X] with a broadcast middle dim (free step 0)."""
    aps = [list(d) for d in ap2d.ap]
    return bass.AP(ap2d.tensor, ap2d.offset, [aps[0], [0, T]] + aps[1:])


def _bc_inner(ap, n):
    """append a 0-step inner dim of size n to an AP."""
    aps = [list(d) for d in ap.ap]
    return bass.AP(ap.tensor, ap.offset, aps + [[0, n]])


def _bc_repl_inner(ap, n):
    """replace a trailing singleton dim with a 0-step dim of size n."""
    aps = [list(d) for d in ap.ap]
    assert aps[-1][1] == 1
    return bass.AP(ap.tensor, ap.offset, aps[:-1] + [[0, n]])


class Cfg:
    def __init__(self, N, E, NC):
        self.N, self.E, self.NC = N, E, NC
        self.IN, self.HID, self.H, self.OUT = 128, 64, 4, 64
        assert N % NC == 0
        self.LOCAL_N = N // NC
        self.NWIN = -(-self.LOCAL_N // 128)
        self.LOCAL_PAD = self.NWIN * 128
        self.NP1 = -(-(N + 1) // 2)  # x pair rows
        self.NP1_PAD = -(-self.NP1 // 128) * 128
        self.NP2_LOC = self.LOCAL_PAD // 2  # T2 pair rows per core
        self.CH = 2  # windows per gather chunk
        self.NCHUNK = -(-self.NWIN // self.CH)
        assert self.NP1_PAD - 1 <= 32767
        assert NC * self.NP2_LOC - 1 <= 32767


class Schedule:
    """Shared (max-over-cores) tile counts + per-core edge orderings."""

    def __init__(self, cfg: Cfg, edge_index: np.ndarray):
        c = cfg
        loop = np.arange(c.N, dtype=np.int64)
        src = np.concatenate([edge_index[0].astype(np.int64), loop])
        dst = np.concatenate([edge_index[1].astype(np.int64), loop])

        owner = dst // c.LOCAL_N
        dloc = dst - owner * c.LOCAL_N
        per_core = []
        for k in range(c.NC):
            m = owner == k
            s, dl = src[m], dloc[m]
            order = np.argsort(dl, kind="stable")
            s, dl = s[order], dl[order]
            w = dl >> 7
            cnt = np.bincount(w, minlength=c.NWIN)
            off = np.concatenate([[0], np.cumsum(cnt)])
            per_core.append((s, dl, off))

        # shared per-window tile counts (max over cores)
        self.TW = [max(-(-int(per_core[k][2][wi + 1] - per_core[k][2][wi])
                        // 128) for k in range(c.NC)) for wi in range(c.NWIN)]
        self.ntiles = sum(self.TW)
        self.TWMAX = max(self.TW)

        # chunk layout: chunk ci covers windows [ci*CH, min(...)+CH)
        self.chunks = []
        t0 = 0
        for ci in range(c.NCHUNK):
            w0 = ci * c.CH
            w1 = min(c.NWIN, w0 + c.CH)
            tiles = sum(self.TW[w0:w1])
            self.chunks.append((w0, w1, t0, tiles))
            t0 += tiles
        assert t0 == self.ntiles
        self.TCMAX = max(t for *_, t in self.chunks)
        assert self.TCMAX * 128 <= 8192

        # per-core padded edge streams in tile order
        self.src_pad = []
        self.dof_pad = []
        for k in range(c.NC):
            s, dl, off = per_core[k]
            sl, dol = [], []
            for wi in range(c.NWIN):
                sw = s[off[wi]:off[wi + 1]]
                dw = dl[off[wi]:off[wi + 1]] & 127
                pad = self.TW[wi] * 128 - len(sw)
                sl.append(np.concatenate(
                    [sw, np.zeros(pad, np.int64)]))
                dol.append(np.concatenate(
                    [dw, np.full(pad, -1, np.int64)]))
            self.src_pad.append(np.concatenate(sl))
            self.dof_pad.append(np.concatenate(dol))


def _wrap_idx(vals: np.ndarray) -> np.ndarray:
    """[n] -> [128, n/16] int16 in dma_gather layout."""
    n = len(vals)
    a = vals.astype(np.int64).reshape(n // 16, 16).T.astype(np.int16)
    return np.tile(a, (8, 1))


def build_core_inputs(cfg: Cfg, sched: Schedule, inputs: dict) -> list:
    c = cfg
    W1 = inputs["W1"].astype(np.float32)
    as1 = inputs["att_src1"].astype(np.float32)
    ad1 = inputs["att_dst1"].astype(np.float32)
    W2 = inputs["W2"].astype(np.float32)
    as2 = inputs["att_src2"].astype(np.float32)
    ad2 = inputs["att_dst2"].astype(np.float32)

    W1h = W1.reshape(c.IN, c.H, c.HID)
    A_s1 = np.einsum("khc,hc->kh", W1h, as1)
    A_d1 = np.einsum("khc,hc->kh", W1h, ad1)
    w1e = np.concatenate([W1, A_s1, A_d1], 1).astype(np.float16)  # [128,264]

    A_s2 = (W2 * as2[0][None, :]).sum(1, keepdims=True)
    A_d2 = (W2 * ad2[0][None, :]).sum(1, keepdims=True)
    w2ext = np.concatenate([W2, A_s2, A_d2], 1).astype(np.float16)  # [256,66]
    w2p = np.concatenate([w2ext[:128], w2ext[128:]], 1)             # [128,132]

    x = inputs["x"].astype(np.float16)
    xpair = np.zeros((c.NP1_PAD, 2 * c.IN), np.float16)
    xpair[: c.N // 2] = x[: (c.N // 2) * 2].reshape(-1, 2 * c.IN)
    if c.N % 2:
        xpair[c.N // 2, : c.IN] = x[-1]

    ident = np.eye(128, dtype=np.float16)
    irep = np.tile(np.arange(128, dtype=np.float16)[None, :], (128, 1))
    icol = np.arange(128, dtype=np.float32)[:, None]
    icoln = -icol
    b1rep = np.tile(inputs["b1"].astype(np.float32)[None, :], (128, 1))
    b2rep = np.tile(inputs["b2"].astype(np.float32)[None, :], (128, 1))

    shared = dict(w1e=w1e, w2p=w2p, xpair=xpair, ident=ident, irep=irep,
                  icol=icol, icoln=icoln, b1rep=b1rep, b2rep=b2rep)

    maps = []
    nt = sched.ntiles
    for k in range(c.NC):
        s = sched.src_pad[k]
        dof = sched.dof_pad[k]
        # L1: pair row + parity
        i1 = (s >> 1).astype(np.int64)
        p1 = (s & 1).astype(np.uint8)
        # L2: owner/local -> pair row + parity
        own = s // c.LOCAL_N
        loc = s - own * c.LOCAL_N
        i2 = own * c.NP2_LOC + (loc >> 1)
        p2 = (loc & 1).astype(np.uint8)

        idx1 = np.zeros((128, nt * 8), np.int16)
        idx2 = np.zeros((128, nt * 8), np.int16)
        for (w0, w1, t0, T) in sched.chunks:
            sl = slice(t0 * 128, (t0 + T) * 128)
            idx1[:, t0 * 8:(t0 + T) * 8] = _wrap_idx(i1[sl])
            idx2[:, t0 * 8:(t0 + T) * 8] = _wrap_idx(i2[sl])

        dofc = dof.reshape(nt, 128).T.astype(np.float16).copy()
        pm2c = p2.reshape(nt, 128).T.copy()
        dofr = np.zeros((c.NCHUNK, 8192), np.float16)
        pmr1 = np.zeros((c.NCHUNK, 8192), np.uint8)
        for ci, (w0, w1, t0, T) in enumerate(sched.chunks):
            n = T * 128
            dofr[ci, :n] = dof[t0 * 128:t0 * 128 + n]
            pmr1[ci, :n] = p1[t0 * 128:t0 * 128 + n]

        xownT = np.zeros((c.IN, c.LOCAL_PAD), np.float16)
        ownx = inputs["x"][k * c.LOCAL_N:(k + 1) * c.LOCAL_N]
        xownT[:, : c.LOCAL_N] = ownx.astype(np.float16).T

        pmsk = np.ones((128, 1), np.float32)
        if c.LOCAL_N % 128:
            pmsk[c.LOCAL_N % 128:] = 0
        m = dict(shared)
        m.update(idx1=idx1, idx2=idx2, dofc=dofc, pm2c=pm2c, dofr=dofr,
                 pmr1=pmr1, xownT=xownT, padmask=pmsk)
        maps.append(m)
    return maps


def build_program(nc: bass.Bass, cfg: Cfg, sched: Schedule):
    c = cfg
    nt = sched.ntiles

    ap = {}
    for name, shape, dt in [
        ("xpair", [c.NP1_PAD, 2 * c.IN], F16),
        ("xownT", [c.IN, c.LOCAL_PAD], F16),
        ("w1e", [128, 264], F16), ("w2p", [128, 132], F16),
        ("ident", [128, 128], F16), ("irep", [128, 128], F16),
        ("icol", [128, 1], F32), ("icoln", [128, 1], F32),
        ("b1rep", [128, 256], F32), ("b2rep", [128, 64], F32),
        ("idx1", [128, nt * 8], I16), ("idx2", [128, nt * 8], I16),
        ("dofc", [128, nt], F16), ("pm2c", [128, nt], U8),
        ("dofr", [c.NCHUNK, 8192], F16), ("pmr1", [c.NCHUNK, 8192], U8),
        ("padmask", [128, 1], F32),
    ]:
        ap[name] = nc.dram_tensor(name, shape, dt, kind="ExternalInput").ap()
    ap_out = nc.dram_tensor("out2", [c.LOCAL_PAD, 64], F32,
                            kind="ExternalOutput").ap()

    with tile.TileContext(nc, num_cores=c.NC) as tc:
        _emit(tc, c, sched, ap, ap_out)
    return nc


def _emit(tc, c: Cfg, sched: Schedule, ap, ap_out):
    nc = tc.nc
    nt = sched.ntiles
    NW = c.NWIN
    TWM = sched.TWMAX
    TCM = sched.TCMAX
    ECM = TCM * 128

    with (
        tc.tile_pool(name="dram", bufs=1, space="DRAM") as dram,
        tc.tile_pool(name="const", bufs=1) as const,
    ):
        T2own = dram.tile([c.NP2_LOC, 256], F16)
        T2full = dram.tile([c.NC * c.NP2_LOC, 256], F16)

        def load_const(name, shape, dt):
            t = const.tile(shape, dt, tag=name, name=name)
            nc.sync.dma_start(out=t[:], in_=ap[name])
            return t

        w1e = load_const("w1e", [128, 264], F16)
        w2p = load_const("w2p", [128, 132], F16)
        ident = load_const("ident", [128, 128], F16)
        irep = load_const("irep", [128, 128], F16)
        icol = load_const("icol", [128, 1], F32)
        icoln = load_const("icoln", [128, 1], F32)
        b1rep = load_const("b1rep", [128, 256], F32)
        b2rep = load_const("b2rep", [128, 64], F32)
        idx1 = load_const("idx1", [128, nt * 8], I16)
        idx2 = load_const("idx2", [128, nt * 8], I16)
        dofc = load_const("dofc", [128, nt], F16)
        pm2c = load_const("pm2c", [128, nt], U8)
        xownT = load_const("xownT", [128, c.LOCAL_PAD], F16)
        padmask = load_const("padmask", [128, 1], F32)
        adw1 = const.tile([128, 4 * NW], F16)
        adw2 = const.tile([128, NW], F16)
        cshift = const.tile([128, 1], F32)
        nc.vector.memset(cshift[:], -C_SHIFT)
        nregs = {}
        for (w0, w1, t0, T) in sched.chunks:
            if T * 128 not in nregs:
                nregs[T * 128] = nc.gpsimd.to_reg(T * 128)
        tc.strict_bb_all_engine_barrier()

        # ---------------- prologue: adw1 (a_d1 of own dsts) ----------------
        with tc.tile_pool(name="pro", bufs=2, space="PSUM") as pro:
            for w in range(NW):
                adps = pro.tile([128, 8], F32, tag="adps")
                nc.tensor.matmul(out=adps[:],
                                 lhsT=xownT[:, w * 128:(w + 1) * 128],
                                 rhs=w1e[:, 256:264], start=True, stop=True)
                nc.vector.tensor_copy(out=adw1[:, 4 * w:4 * w + 4],
                                      in_=adps[:, 4:8])

        # ---------------- layer 1 ----------------
        with (
            tc.tile_pool(name="xg", bufs=2) as xgp,
            tc.tile_pool(name="oh", bufs=2) as ohp,
            tc.tile_pool(name="rep", bufs=2) as repp,
            tc.tile_pool(name="hsb", bufs=2) as hsbp,
            tc.tile_pool(name="small", bufs=3) as small,
            tc.tile_pool(name="stg", bufs=3) as stg,
            tc.tile_pool(name="psH", bufs=2, space="PSUM") as psHp,
            tc.tile_pool(name="psN", bufs=2, space="PSUM") as psNp,
            tc.tile_pool(name="psA", bufs=1, space="PSUM") as psAp,
            tc.tile_pool(name="psT", bufs=1, space="PSUM") as psTp,
        ):
            for (ci, (w0, w1c, t0, Tc)) in enumerate(sched.chunks):
                Ec = Tc * 128
                xg = xgp.tile([128, 2, Ec], F16, tag="xg")
                nc.gpsimd.dma_gather(xg[:, :, :], ap["xpair"],
                                     idx1[:, t0 * 8:(t0 + Tc) * 8],
                                     Ec, nregs[Ec], 2 * c.IN,
                                     transpose=True, single_packet=False)
                dofrep = repp.tile([128, ECM], F16, tag="dofrep")
                drsrc = bass.AP(ap["dofr"].tensor,
                                ap["dofr"][ci:ci + 1, 0:Ec].offset,
                                [[0, 128], [1, Ec]])
                nc.sync.dma_start(out=dofrep[:, 0:Ec], in_=drsrc)
                pmrep = repp.tile([128, ECM], U8, tag="pmrep")
                pmsrc = bass.AP(ap["pmr1"].tensor,
                                ap["pmr1"][ci:ci + 1, 0:Ec].offset,
                                [[0, 128], [1, Ec]])
                nc.sync.dma_start(out=pmrep[:, 0:Ec], in_=pmsrc)

                # parity select: xsel = parity ? odd half : even half
                nc.vector.copy_predicated(out=xg[:, 0, :],
                                          mask=pmrep[:, 0:Ec],
                                          data=xg[:, 1, :])
                # ohT on ACT: relu(1 - (dof - d)^2)
                ohT = ohp.tile([128, ECM], F16, tag="ohT")
                nc.scalar.activation(out=ohT[:, 0:Ec], in_=dofrep[:, 0:Ec],
                                     func=ACT.Square, bias=icoln[:, :])
                nc.scalar.activation(out=ohT[:, 0:Ec], in_=ohT[:, 0:Ec],
                                     func=ACT.Relu, scale=-1.0, bias=1.0)
                # oh on DVE: [e, T, dst]
                oh = ohp.tile([128, TCM, 128], F16, tag="oh")
                nc.vector.tensor_tensor(
                    out=oh[:, 0:Tc, :],
                    in0=dofc[:, t0:t0 + Tc].to_broadcast([128, Tc, 128]),
                    in1=_midb(irep[:, :], Tc),
                    op=OP.is_equal)

                tw = t0
                for w in range(w0, w1c):
                    T = sched.TW[w]
                    e0t = tw - t0
                    e0 = e0t * 128
                    psNT = psNp.tile([128, 260], F32, tag="psNT")
                    hsb = hsbp.tile([128, TWM, 264], F16, tag="hsb")
                    adp = psAp.tile([128, TWM * 4], F32, tag="adp")
                    for t in range(T):
                        psH = psHp.tile([128, 264], F32, tag="psH")
                        nc.tensor.matmul(
                            out=psH[:],
                            lhsT=xg[:, 0, e0 + t * 128:e0 + (t + 1) * 128],
                            rhs=w1e[:], start=True, stop=True)
                        nc.scalar.copy(out=hsb[:, t, :], in_=psH[:])
                        nc.tensor.matmul(
                            out=adp[:, t * 4:(t + 1) * 4],
                            lhsT=ohT[:, e0 + t * 128:e0 + (t + 1) * 128],
                            rhs=adw1[:, 4 * w:4 * w + 4],
                            start=True, stop=True)
                    # batched attention scalars for the window
                    ea = small.tile([128, TWM * 4], F32, tag="ea")
                    nc.vector.tensor_tensor(
                        out=ea[:, 0:T * 4].rearrange("p (t h) -> p t h", t=T),
                        in0=hsb[:, 0:T, 256:260],
                        in1=adp[:, 0:T * 4].rearrange("p (t h) -> p t h", t=T),
                        op=OP.add)
                    pos = small.tile([128, TWM * 4], F32, tag="pos")
                    nc.vector.tensor_scalar(out=pos[:, 0:T * 4],
                                            in0=ea[:, 0:T * 4], scalar1=0.0,
                                            scalar2=None, op0=OP.max)
                    nc.vector.tensor_scalar(out=ea[:, 0:T * 4],
                                            in0=ea[:, 0:T * 4], scalar1=0.0,
                                            scalar2=NEG_SLOPE, op0=OP.min,
                                            op1=OP.mult)
                    nc.vector.tensor_tensor(out=ea[:, 0:T * 4],
                                            in0=ea[:, 0:T * 4],
                                            in1=pos[:, 0:T * 4], op=OP.add)
                    nc.scalar.activation(
                        out=hsb[:, 0:T, 256:260],
                        in_=ea[:, 0:T * 4].rearrange("p (t h) -> p t h", t=T),
                        func=ACT.Exp, bias=cshift[:, :])
                    # alpha * h (4D broadcast of the 4 exp values over 64)
                    nc.vector.tensor_tensor(
                        out=hsb[:, 0:T, 0:256].rearrange(
                            "p t (h f) -> p t h f", h=4),
                        in0=hsb[:, 0:T, 0:256].rearrange(
                            "p t (h f) -> p t h f", h=4),
                        in1=_bc_inner(hsb[:, 0:T, 256:260], 64),
                        op=OP.mult)
                    for t in range(T):
                        nc.tensor.matmul(
                            out=psNT[:],
                            lhsT=oh[:, e0t + t, :],
                            rhs=hsb[:, t, 0:260],
                            start=(t == 0), stop=(t == T - 1))
                    # ---- window epilogue ----
                    sbN = small.tile([128, 260], F32, tag="sbN")
                    nc.vector.tensor_copy(out=sbN[:], in_=psNT[:])
                    recD = small.tile([128, 4], F32, tag="recD")
                    # pad dsts have empty segments: clamp 0 -> tiny
                    nc.vector.tensor_scalar(out=sbN[:, 256:260],
                                            in0=sbN[:, 256:260],
                                            scalar1=1e-20, scalar2=None,
                                            op0=OP.max)
                    nc.vector.reciprocal(out=recD[:], in_=sbN[:, 256:260])
                    nrm = small.tile([128, 256], F32, tag="nrm")
                    nc.vector.tensor_tensor(
                        out=nrm[:].rearrange("p (h f) -> p h f", h=4),
                        in0=sbN[:, 0:256].rearrange("p (h f) -> p h f", h=4),
                        in1=_bc_inner(recD[:, :], 64),
                        op=OP.mult)
                    nc.vector.tensor_tensor(out=nrm[:], in0=nrm[:],
                                            in1=b1rep[:], op=OP.add)
                    ex1 = small.tile([128, 256], F32, tag="ex1")
                    nc.scalar.activation(out=ex1[:], in_=nrm[:], func=ACT.Exp)
                    nc.scalar.activation(out=ex1[:], in_=ex1[:],
                                         func=ACT.Relu, scale=-1.0, bias=1.0)
                    nc.vector.tensor_scalar(out=nrm[:], in0=nrm[:],
                                            scalar1=0.0, scalar2=None,
                                            op0=OP.max)
                    elu = stg.tile([128, 256], F16, tag="elu")
                    nc.vector.tensor_tensor(out=elu[:], in0=nrm[:],
                                            in1=ex1[:], op=OP.subtract)
                    if w == NW - 1 and c.LOCAL_N % 128:
                        nc.vector.tensor_scalar(out=elu[:], in0=elu[:],
                                                scalar1=padmask[:, :],
                                                scalar2=None, op0=OP.mult)
                    # h2 = elu @ W2ext  (transpose elu blocks for lhsT)
                    pst2 = psTp.tile([128, 66], F32, tag="pst2")
                    for fb in range(2):
                        ptr = psTp.tile([128, 128], F16, tag="ptr")
                        nc.tensor.transpose(
                            out=ptr[:], in_=elu[:, fb * 128:(fb + 1) * 128],
                            identity=ident[:])
                        eT = stg.tile([128, 128], F16, tag=f"eT{fb}",
                                      name=f"eT{fb}")
                        nc.vector.tensor_copy(out=eT[:], in_=ptr[:])
                        nc.tensor.matmul(out=pst2[:], lhsT=eT[:],
                                         rhs=w2p[:, fb * 66:(fb + 1) * 66],
                                         start=(fb == 0), stop=(fb == 1))
                    t2s = stg.tile([128, 128], F16, tag="t2s")
                    nc.gpsimd.memset(t2s[:, 66:128], 0.0)
                    nc.vector.tensor_copy(out=t2s[:, 0:66], in_=pst2[:])
                    nc.vector.tensor_copy(out=adw2[:, w:w + 1],
                                          in_=pst2[:, 65:66])
                    # pair rows are contiguous: node p of the window lands at
                    # byte offset p*256B within the window's 64 pair rows
                    nc.sync.dma_start(
                        out=T2own[w * 64:(w + 1) * 64, :].rearrange(
                            "a (b r) -> (a b) r", b=2),
                        in_=t2s[:])
                    tw += T

            nc.gpsimd.collective_compute(
                "AllGather", OP.bypass,
                replica_groups=[list(range(c.NC))],
                ins=[T2own.opt()], outs=[T2full.opt()])

        # ---------------- layer 2 ----------------
        with (
            tc.tile_pool(name="g2", bufs=2) as g2p,
            tc.tile_pool(name="oh2", bufs=2) as ohp2,
            tc.tile_pool(name="rep2", bufs=2) as repp2,
            tc.tile_pool(name="sm2", bufs=3) as small2,
            tc.tile_pool(name="psO", bufs=2, space="PSUM") as psOp,
            tc.tile_pool(name="psA2", bufs=2, space="PSUM") as psAp2,
        ):
            for (ci, (w0, w1c, t0, Tc)) in enumerate(sched.chunks):
                Ec = Tc * 128
                g2 = g2p.tile([128, TCM, 256], F16, tag="g2")
                nc.gpsimd.dma_gather(g2[:, 0:Tc, :], T2full,
                                     idx2[:, t0 * 8:(t0 + Tc) * 8],
                                     Ec, nregs[Ec], 256,
                                     single_packet=False)
                dofrep = repp2.tile([128, ECM], F16, tag="dofrep2")
                drsrc = bass.AP(ap["dofr"].tensor,
                                ap["dofr"][ci:ci + 1, 0:Ec].offset,
                                [[0, 128], [1, Ec]])
                nc.sync.dma_start(out=dofrep[:, 0:Ec], in_=drsrc)
                # parity select: [128:194] -> [0:66]
                nc.vector.copy_predicated(
                    out=g2[:, 0:Tc, 0:66],
                    mask=_bc_inner(pm2c[:, t0:t0 + Tc], 66),
                    data=g2[:, 0:Tc, 128:194])
                ohT = ohp2.tile([128, ECM], F16, tag="ohT2")
                nc.scalar.activation(out=ohT[:, 0:Ec], in_=dofrep[:, 0:Ec],
                                     func=ACT.Square, bias=icoln[:, :])
                nc.scalar.activation(out=ohT[:, 0:Ec], in_=ohT[:, 0:Ec],
                                     func=ACT.Relu, scale=-1.0, bias=1.0)
                oh = ohp2.tile([128, TCM, 128], F16, tag="oh2")
                nc.vector.tensor_tensor(
                    out=oh[:, 0:Tc, :],
                    in0=dofc[:, t0:t0 + Tc].to_broadcast([128, Tc, 128]),
                    in1=_midb(irep[:, :], Tc),
                    op=OP.is_equal)

                tw = t0
                for w in range(w0, w1c):
                    T = sched.TW[w]
                    e0t = tw - t0
                    psO = psOp.tile([128, 65], F32, tag="psO")
                    adp = psAp2.tile([128, TWM], F32, tag="adp2")
                    for t in range(T):
                        nc.tensor.matmul(
                            out=adp[:, t:t + 1],
                            lhsT=ohT[:, (e0t + t) * 128:(e0t + t + 1) * 128],
                            rhs=adw2[:, w:w + 1], start=True, stop=True)
                    ea = small2.tile([128, TWM], F32, tag="ea2")
                    nc.vector.tensor_tensor(
                        out=ea[:, 0:T],
                        in0=g2[:, e0t:e0t + T, 64:65].rearrange(
                            "p t o -> p (t o)"),
                        in1=adp[:, 0:T], op=OP.add)
                    pos = small2.tile([128, TWM], F32, tag="pos2")
                    nc.vector.tensor_scalar(out=pos[:, 0:T], in0=ea[:, 0:T],
                                            scalar1=0.0, scalar2=None,
                                            op0=OP.max)
                    nc.vector.tensor_scalar(out=ea[:, 0:T], in0=ea[:, 0:T],
                                            scalar1=0.0, scalar2=NEG_SLOPE,
                                            op0=OP.min, op1=OP.mult)
                    nc.vector.tensor_tensor(out=ea[:, 0:T], in0=ea[:, 0:T],
                                            in1=pos[:, 0:T], op=OP.add)
                    nc.scalar.activation(
                        out=g2[:, e0t:e0t + T, 64:65].rearrange(
                            "p t o -> p (t o)"),
                        in_=ea[:, 0:T], func=ACT.Exp, bias=cshift[:, :])
                    nc.vector.tensor_tensor(
                        out=g2[:, e0t:e0t + T, 0:64],
                        in0=g2[:, e0t:e0t + T, 0:64],
                        in1=_bc_repl_inner(g2[:, e0t:e0t + T, 64:65], 64),
                        op=OP.mult)
                    for t in range(T):
                        nc.tensor.matmul(out=psO[:],
                                         lhsT=oh[:, e0t + t, :],
                                         rhs=g2[:, e0t + t, 0:65],
                                         start=(t == 0), stop=(t == T - 1))
                    # epilogue
                    sbO = small2.tile([128, 65], F32, tag="sbO")
                    nc.vector.tensor_copy(out=sbO[:], in_=psO[:])
                    rec2 = small2.tile([128, 1], F32, tag="rec2")
                    nc.vector.tensor_scalar(out=sbO[:, 64:65],
                                            in0=sbO[:, 64:65],
                                            scalar1=1e-20, scalar2=None,
                                            op0=OP.max)
                    nc.vector.reciprocal(out=rec2[:], in_=sbO[:, 64:65])
                    o = small2.tile([128, 64], F32, tag="o")
                    nc.vector.tensor_scalar(out=o[:], in0=sbO[:, 0:64],
                                            scalar1=rec2[:, :], scalar2=None,
                                            op0=OP.mult)
                    nc.vector.tensor_tensor(out=o[:], in0=o[:], in1=b2rep[:],
                                            op=OP.add)
                    nc.sync.dma_start(out=ap_out[w * 128:(w + 1) * 128, :],
                                      in_=o[:])
                    tw += T


def kernel(x, edge_index, W1, att_src1, att_dst1, b1, W2, att_src2, att_dst2,
           b2) -> np.ndarray:
    inputs = dict(x=np.asarray(x), edge_index=np.asarray(edge_index),
                  W1=np.asarray(W1), att_src1=np.asarray(att_src1),
                  att_dst1=np.asarray(att_dst1), b1=np.asarray(b1),
                  W2=np.asarray(W2), att_src2=np.asarray(att_src2),
                  att_dst2=np.asarray(att_dst2), b2=np.asarray(b2))
    cfg = Cfg(N=inputs["x"].shape[0], E=inputs["edge_index"].shape[1], NC=8)
    sched = Schedule(cfg, inputs["edge_index"])
    in_maps = build_core_inputs(cfg, sched, inputs)

    nc = bacc.Bacc("TRN2", target_bir_lowering=False, debug=False,
                   num_devices=cfg.NC)
    build_program(nc, cfg, sched)
    nc.compile()

    import os
    trace = bool(int(os.environ.get("GAT_TRACE", "0")))
    res = bass_utils.run_bass_kernel_spmd(nc, in_maps,
                                          core_ids=list(range(cfg.NC)),
                                          trace=trace)
    kernel.last_exec_time_ns = res.exec_time_ns
    kernel.last_trace = res.instructions_and_trace
    out = np.concatenate(
        [res.results[k]["out2"][: cfg.LOCAL_N] for k in range(cfg.NC)], 0)
    return out.astype(np.float32)


if __name__ == "__main__":
    from ref_numpy import get_inputs

    inputs = get_inputs()
    out = kernel(**inputs)
    expected = np.load("/tmp/expected_np.npy")
    err = np.abs(out - expected)
    print("abs max err %.3e  rel %.3e" % (err.max(),
                                          err.max() / np.abs(expected).max()))
